# revision 1
# baseline (speedup 1.0000x reference)
"""GroupNorm + single-head self-attention block (B=16, C=512, H=W=32) on 8
TRN2 NeuronCores.

Sharding: pure data-parallel over batch — 2 samples per core, no collectives.

Per-sample dataflow (C=512 channels, N=1024 pixels), everything laid out
channels-on-partitions so no transposes are ever needed:

  x   [c, n]   4 tiles [128, 1024]
  GN: per-channel mean/var via bn_stats, group (16-ch) aggregation via a
      tiny matmul against a group-indicator matrix, scatter back the same
      way; h = a_c * x - b_c  (per-partition scalars).
  Q = wq @ h -> [o, n]  (lhsT = wqT tiles, rhs = h)          + bq
  K = wk @ h -> [o, m]                                        + bk
  V [m, c] = h_tile.T @ wvT   (h as the stationary operand -> V lands
      pixel-major; bias bv folded into c0 = wo @ bv + bo at the end)
  ST [m, n] = K_tile.T @ Q    (scores, transposed layout)
  E = exp(ST / sqrt(C))       (no max-subtraction: scores are O(5))
  R [128, n] = 1 / (ones128.T @ E)   (softmax denominators, accumulated
      directly in broadcast form on the PE; one reciprocal per chunk)
  OT [c, n] = V_tile.T @ E    (un-normalized attn output, channel-major)
  OT *= R                     (rides the PSUM evacuation)
  o2 [o, n] = woT_tile.T @ OT
  y = x + o2 + c0[o]          (single fused scalar_tensor_tensor)

Matmuls run in bf16 (fp32 PSUM accumulation; CoreSim-validated rel err
3.5e-4 vs the fp32 reference). Emission is phase-major across the two
samples so one sample's matmuls fill the other's dependency stalls;
small constants ride a single packed DMA; weights load behind x on the
HWDGE queue; outputs drain via the GPSIMD SWDGE path.
"""

import numpy as np

import concourse.bass as bass
import concourse.mybir as mybir
from concourse import tile
from concourse.bass_utils import run_bass_kernel_spmd


def _install_drain_patch():
    """This walrus build rejects Drain instructions carrying more than one
    semaphore wait (setupSyncWait<CTRL_NO_STRUCT>). Split the TileContext
    tail drain's waits across a chain of single-wait drains."""
    import concourse.tile as tile_mod
    from concourse.vector_clock import ScopedClock

    if getattr(tile_mod.TileContext, "_drain_patch_installed", False):
        return

    def _patched(self, tick_clock, wait_clock):
        nc = self.nc
        drain_inst = nc.sync.drain()
        wait_clock.add_sem_waits(
            drain_inst.ins, ScopedClock({None: tick_clock.global_clock})
        )
        si = drain_inst.ins.sync_info
        waits = list(si.on_wait or []) if si is not None else []
        if len(waits) > 1:
            si.on_wait = waits[:1]
            for w in waits[1:]:
                extra = nc.sync.drain()
                extra.ins.sync_info = mybir.SyncInfo(on_wait=[w], on_update=[])

        nc.all_engine_barrier()
        assert self.sems is not None
        popped = nc._tile_sem_poison_stack.pop()
        assert popped is self._sem_poison
        nc.clear_and_free_semaphores(list(self.sems.allocated().values()))
        nc.all_engine_barrier()

    tile_mod.TileContext._drain_and_barrier = _patched
    tile_mod.TileContext._drain_patch_installed = True


_install_drain_patch()

F32 = mybir.dt.float32
F32R = mybir.dt.float32r
BF16 = mybir.dt.bfloat16

B, C, H, W = 16, 512, 32, 32
N = H * W                      # 1024 pixels
NCORES = 8
S = B // NCORES                # samples per core
CT = C // 128                  # 4 channel tiles
NW = 512                       # matmul moving-operand chunk (fp32r max)
NCH = N // NW                  # 2 chunks
MT = N // 128                  # 8 pixel tiles
GROUPS = 32
GSIZE = C // GROUPS            # 16 channels per group
GPT = 128 // GSIZE             # 8 groups per channel tile
EPS = 1e-5

COMPUTE = "bf16"               # "f32r" | "bf16" | "f32"

CDT = {"bf16": BF16, "f32r": F32R, "f32": F32}[COMPUTE]


def _cc(ap):
    """Cast an AP for TensorEngine consumption (tiles already carry the
    compute dtype; this is now a no-op kept for clarity)."""
    return ap


def _split_waits(nc, maxw=1):
    """This walrus build caps the number of sync waits an instruction can
    carry (varies by instruction class; Drain and Matmult/LDWEIGHTS observed
    failing). Hoist excess waits onto standalone EventSemaphore instructions
    inserted just before, on the same engine."""
    cnt = 0
    for f in nc.m.functions:
        for bb in f.blocks:
            insts = list(bb.instructions)
            out = []
            changed = False
            for inst in insts:
                si = inst.sync_info
                waits = list(si.on_wait) if (si is not None and si.on_wait) else []
                if len(waits) > maxw:
                    for w in waits[:-maxw]:
                        ev = mybir.InstEventSemaphore(
                            name=f"waitsplit_{cnt}", ins=[], outs=[])
                        cnt += 1
                        ev.engine = inst.engine
                        ev.sync_info = mybir.SyncInfo(on_wait=[w], on_update=[])
                        out.append(ev)
                    si.on_wait = waits[-maxw:]
                    changed = True
                out.append(inst)
            if changed:
                _replace_block_instructions(bb, out)
    return cnt


def _replace_block_instructions(bb, insts):
    try:
        bb.instructions = insts
        return
    except Exception:
        pass
    try:
        bb.instructions.clear()
        for i in insts:
            bb.instructions.append(i)
        return
    except Exception:
        pass
    raise RuntimeError("cannot rewrite block instructions")


def build_nc(split_waits=True):
    nc = bass.Bass(target_bir_lowering=False)

    x_ext = nc.declare_dram_parameter("x", [S, CT, 128, N], F32, isOutput=False)
    w_ext = {}
    for w in ("wq", "wk", "wv", "wo"):
        w_ext[w] = nc.declare_dram_parameter(w, [CT, 128, C], CDT, isOutput=False)
    b_ext = {}
    # cblob columns: bq[4] bk[4] c0[4] gnw[4] gnb[4] gmat[8] -> [128, 28] f32
    b_ext["cblob"] = nc.declare_dram_parameter("cblob", [128, 28], F32,
                                               isOutput=False)
    b_ext["gmt"] = nc.declare_dram_parameter("gmt", [GPT, 128], F32,
                                             isOutput=False)
    b_ext["ones2"] = nc.declare_dram_parameter("ones2", [128, 128], CDT,
                                               isOutput=False)
    out_ext = nc.declare_dram_parameter("out", [S, CT, 128, N], F32, isOutput=True)

    with tile.TileContext(nc) as tc:
        _body(nc, tc, x_ext, w_ext, b_ext, out_ext)
    if split_waits:
        _split_waits(nc)
    return nc


def _body(nc, tc, x_ext, w_ext, b_ext, out_ext):
    import contextlib

    ctx = contextlib.ExitStack()
    with ctx:
        consts = ctx.enter_context(tc.tile_pool(name="consts", bufs=1))
        sb = ctx.enter_context(tc.tile_pool(name="sb", bufs=1))
        ps = ctx.enter_context(tc.tile_pool(name="ps", space="PSUM", bufs=1))

        # ---------------- constants ----------------
        # Weight tiles are allocated now but their DMAs are emitted after the
        # x loads (phase_weights) so x wins the head-of-line on HWDGE.
        w_sb = {}
        for w in ("wq", "wk", "wv", "wo"):
            w_sb[w] = [
                consts.tile([128, C], CDT, name=f"{w}_{ct}", tag=f"{w}_{ct}")
                for ct in range(CT)
            ]

        def phase_weights(names):
            for w in names:
                for ct in range(CT):
                    nc.sync.dma_start(out=w_sb[w][ct], in_=w_ext[w][ct])
        cblob = consts.tile([128, 28], F32, tag="cblob")
        nc.gpsimd.dma_start(out=cblob, in_=b_ext["cblob"][:, :])
        b_sb = {}
        for bi, b in enumerate(("bq", "bk", "c0", "gnw", "gnb")):
            b_sb[b] = [cblob[:, bi * CT + ct:bi * CT + ct + 1]
                       for ct in range(CT)]

        ones2 = consts.tile([128, 128], CDT, tag="ones2")
        nc.gpsimd.dma_start(out=ones2, in_=b_ext["ones2"][:, :])
        warm = ps.tile([128, NW], F32, tag="small", bufs=2)
        for wi in range(12):
            nc.tensor.matmul(warm[:, 0:128], ones2, ones2,
                             start=(wi == 0), stop=(wi == 11))

        # Group-indicator matrices.
        # gmat[c, g] = 1/GSIZE where c // GSIZE == g   (gather:  [128, GPT])
        # gmt [g, c] = 1       where c // GSIZE == g   (scatter: [GPT, 128])
        gmat = cblob[:, 20:20 + GPT]
        gmt = consts.tile([GPT, 128], F32, tag="gmt")
        nc.gpsimd.dma_start(out=gmt, in_=b_ext["gmt"][:, :])

        eps_g = consts.tile([GPT, 1], F32, tag="eps_g")
        nc.vector.memset(eps_g, EPS)

        # c0[o] = (wo @ bv)[o] + bo[o], precomputed on the host — folds the
        # V bias exactly: after softmax-normalization the bv term contributes
        # bv broadcast through wo (attention rows sum to 1).
        c0_sb = b_sb["c0"]

        # ---------------- per-sample pipelines, emitted phase-major ----------------
        # Emitting each phase for both samples back-to-back lets the PE fill
        # one sample's dependency stalls (GroupNorm chain, softmax-denominator
        # chain) with the other sample's matmuls.
        inv_sqrt_c = 1.0 / float(np.sqrt(C))
        st = [dict() for _ in range(S)]

        def phase_load(s):
            x_sb = []
            for ct in range(CT):
                xt = sb.tile([128, N], F32, name=f"x{s}_{ct}", tag=f"x_{ct}",
                             bufs=2)
                for q in range(2):
                    nc.sync.dma_start(out=xt[:, q * 512:(q + 1) * 512],
                                      in_=x_ext[s, ct, :, q * 512:(q + 1) * 512])
                x_sb.append(xt)
            st[s]["x"] = x_sb

        def phase_gn(s):
            x_sb = st[s]["x"]
            stats3 = []
            for ct in range(CT):
                s3 = sb.tile([128, 3], F32, tag=f"s3_{ct}", bufs=2)
                if ct < 2:
                    # DVE path: bn_stats -> [mean, var], plus mean^2
                    st6 = sb.tile([128, 2, 6], F32, tag="st6", bufs=4)
                    nc.vector.bn_stats(out=st6[:, 0, :], in_=x_sb[ct][:, 0:512])
                    nc.vector.bn_stats(out=st6[:, 1, :],
                                       in_=x_sb[ct][:, 512:1024])
                    nc.vector.bn_aggr(out=s3[:, 0:2], in_=st6)
                    nc.vector.tensor_mul(out=s3[:, 2:3], in0=s3[:, 0:1],
                                         in1=s3[:, 0:1])
                else:
                    # ACT path: accum_out sums along the free axis.
                    # col0 = mean (scale 1/N), col1 = E[x^2] ((x/sqrt(N))^2),
                    # col2 = 0.  Downstream uses col1+col2 = E[x^2], same as
                    # var + mean^2 on the DVE path.
                    scr = sb.tile([128, N], CDT, tag="gnscr", bufs=2)
                    nc.scalar.activation(
                        out=scr, in_=x_sb[ct],
                        func=mybir.ActivationFunctionType.Copy,
                        scale=1.0 / N, accum_out=s3[:, 0:1])
                    nc.scalar.activation(
                        out=scr, in_=x_sb[ct],
                        func=mybir.ActivationFunctionType.Square,
                        scale=1.0 / float(np.sqrt(N)), accum_out=s3[:, 1:2])
                    nc.vector.memset(s3[:, 2:3], 0.0)
                stats3.append(s3)

            h_sb = [None] * CT
            for ct in range(CT):
                gp = ps.tile([GPT, 3], F32, tag="small", bufs=2)
                nc.tensor.matmul(gp, gmat, stats3[ct], start=True, stop=True)
                gs = sb.tile([GPT, 3], F32, tag="gs", bufs=4)
                nc.vector.tensor_copy(out=gs, in_=gp)
                # var_g = (E[var] + E[mean^2]) - (E[mean])^2
                m2 = sb.tile([GPT, 3], F32, tag="m2", bufs=4)
                nc.vector.tensor_add(out=m2[:, 1:2], in0=gs[:, 1:2],
                                     in1=gs[:, 2:3])
                nc.vector.tensor_mul(out=m2[:, 0:1], in0=gs[:, 0:1],
                                     in1=gs[:, 0:1])
                nc.vector.tensor_sub(out=m2[:, 2:3], in0=m2[:, 1:2],
                                     in1=m2[:, 0:1])
                s2 = sb.tile([GPT, 2], F32, tag="s2", bufs=4)
                nc.scalar.activation(out=s2[:, 1:2], in_=m2[:, 2:3],
                                     func=mybir.ActivationFunctionType.Sqrt,
                                     bias=eps_g, scale=1.0)
                nc.gpsimd.tensor_copy(out=s2[:, 0:1], in_=gs[:, 0:1])
                nc.vector.reciprocal(out=s2[:, 1:2], in_=s2[:, 1:2])

                abp = ps.tile([128, 2], F32, tag="small", bufs=2)
                nc.tensor.matmul(abp, gmt, s2, start=True, stop=True)
                a_c = sb.tile([128, 1], F32, tag=f"a_{ct}", bufs=2)
                nc.vector.tensor_mul(out=a_c, in0=abp[:, 1:2],
                                     in1=b_sb["gnw"][ct])
                bneg = sb.tile([128, 1], F32, tag=f"bneg_{ct}", bufs=2)
                nc.vector.scalar_tensor_tensor(
                    out=bneg, in0=abp[:, 0:1], scalar=a_c,
                    in1=b_sb["gnb"][ct],
                    op0=mybir.AluOpType.mult, op1=mybir.AluOpType.subtract,
                )
                ht = sb.tile([128, N], CDT, name=f"h{s}_{ct}", tag=f"h_{ct}",
                             bufs=2)
                eng = nc.vector if ct < 2 else nc.gpsimd
                eng.tensor_scalar(
                    out=ht, in0=x_sb[ct], scalar1=a_c, scalar2=bneg,
                    op0=mybir.AluOpType.mult, op1=mybir.AluOpType.subtract,
                )
                h_sb[ct] = ht
            st[s]["h"] = h_sb

        def phase_qkv(s):
            h_sb = st[s]["h"]
            q_sb, k_sb = [], []
            for name, wt, bias, dst in (("q", "wq", "bq", q_sb),
                                        ("k", "wk", "bk", k_sb)):
                for ot in range(CT):
                    t = sb.tile([128, N], CDT, name=f"{name}{s}_{ot}",
                                tag=f"{name}_{ot}", bufs=2)
                    dst.append(t)
                    for nch in range(NCH):
                        pp = ps.tile([128, NW], F32, tag="mm", bufs=6)
                        for ct in range(CT):
                            nc.tensor.matmul(
                                pp,
                                _cc(w_sb[wt][ct][:, ot * 128:(ot + 1) * 128]),
                                _cc(h_sb[ct][:, nch * NW:(nch + 1) * NW]),
                                start=(ct == 0), stop=(ct == CT - 1),
                            )
                        nc.vector.tensor_scalar_add(
                            out=t[:, nch * NW:(nch + 1) * NW], in0=pp,
                            scalar1=b_sb[bias][ot],
                        )
            v_sb = []
            for mt in range(MT):
                vt = sb.tile([128, C], CDT, name=f"v{s}_{mt}", tag=f"v_{mt}",
                             bufs=2)
                vp = ps.tile([128, NW], F32, tag="mm", bufs=6)
                for ct in range(CT):
                    nc.tensor.matmul(
                        vp,
                        _cc(h_sb[ct][:, mt * 128:(mt + 1) * 128]),
                        _cc(w_sb["wv"][ct]),
                        start=(ct == 0), stop=(ct == CT - 1),
                    )
                nc.scalar.copy(out=vt, in_=vp)
                v_sb.append(vt)
            st[s]["q"], st[s]["k"], st[s]["v"] = q_sb, k_sb, v_sb

        def phase_st(s):
            q_sb, k_sb = st[s]["q"], st[s]["k"]
            e_sb = [sb.tile([128, N], CDT, name=f"e{s}_{mt}", tag=f"e_{mt}",
                            bufs=2) for mt in range(MT)]
            # nch outer: all of chunk 0's scores+exp land first, so the
            # softmax-denominator accumulation for chunk 0 overlaps chunk 1.
            for nch in range(NCH):
                for mt in range(MT):
                    sp = ps.tile([128, NW], F32, tag="mm", bufs=6)
                    for ct in range(CT):
                        nc.tensor.matmul(
                            sp,
                            _cc(k_sb[ct][:, mt * 128:(mt + 1) * 128]),
                            _cc(q_sb[ct][:, nch * NW:(nch + 1) * NW]),
                            start=(ct == 0), stop=(ct == CT - 1),
                        )
                    nc.scalar.activation(
                        out=e_sb[mt][:, nch * NW:(nch + 1) * NW], in_=sp,
                        func=mybir.ActivationFunctionType.Exp,
                        scale=inv_sqrt_c,
                    )
            st[s]["e"] = e_sb

        def phase_sr(s):
            e_sb = st[s]["e"]
            # R[p, n] = 1 / sum_m E[m, n], built directly in broadcast form:
            # ones2.T @ E accumulates the column sums into every partition.
            R_sb = sb.tile([128, N], F32, tag="R", bufs=2)
            for nch in range(NCH):
                srp = ps.tile([128, NW], F32, tag="small", bufs=2)
                for mt in range(MT):
                    nc.tensor.matmul(
                        srp, _cc(ones2),
                        _cc(e_sb[mt][:, nch * NW:(nch + 1) * NW]),
                        start=(mt == 0), stop=(mt == MT - 1),
                    )
                nc.vector.reciprocal(out=R_sb[:, nch * NW:(nch + 1) * NW],
                                     in_=srp)
            st[s]["R"] = R_sb

        def phase_ot(s):
            v_sb, e_sb, R_sb = st[s]["v"], st[s]["e"], st[s]["R"]
            ot_sb = [sb.tile([128, N], CDT, name=f"ot{s}_{ct}", tag=f"ot_{ct}",
                             bufs=2) for ct in range(CT)]
            # nch outer: chunk 0's four OT tiles finish first, so the o2
            # projection for chunk 0 starts half a phase earlier.
            for nch in range(NCH):
                for ct in range(CT):
                    op_ = ps.tile([128, NW], F32, tag="mm", bufs=6)
                    for mt in range(MT):
                        nc.tensor.matmul(
                            op_,
                            _cc(v_sb[mt][:, ct * 128:(ct + 1) * 128]),
                            _cc(e_sb[mt][:, nch * NW:(nch + 1) * NW]),
                            start=(mt == 0), stop=(mt == MT - 1),
                        )
                    nc.vector.tensor_mul(
                        out=ot_sb[ct][:, nch * NW:(nch + 1) * NW], in0=op_,
                        in1=R_sb[:, nch * NW:(nch + 1) * NW],
                    )
            st[s]["ot"] = ot_sb

        def phase_o2(s):
            x_sb, ot_sb = st[s]["x"], st[s]["ot"]
            for nch in range(NCH):
                for ot in range(CT):
                    o2p = ps.tile([128, NW], F32, tag="mm", bufs=6)
                    for ct in range(CT):
                        nc.tensor.matmul(
                            o2p,
                            _cc(w_sb["wo"][ct][:, ot * 128:(ot + 1) * 128]),
                            _cc(ot_sb[ct][:, nch * NW:(nch + 1) * NW]),
                            start=(ct == 0), stop=(ct == CT - 1),
                        )
                    # y = (o2 + c0) + x, written in place over x
                    nc.vector.scalar_tensor_tensor(
                        out=x_sb[ot][:, nch * NW:(nch + 1) * NW], in0=o2p,
                        scalar=c0_sb[ot],
                        in1=x_sb[ot][:, nch * NW:(nch + 1) * NW],
                        op0=mybir.AluOpType.add, op1=mybir.AluOpType.add,
                    )
                    nc.gpsimd.dma_start(
                        out=out_ext[s, ot, :, nch * NW:(nch + 1) * NW],
                        in_=x_sb[ot][:, nch * NW:(nch + 1) * NW])

        # x(s0) first (feeds GroupNorm), then the weights QKV needs first,
        # then x(s1), then the rest — keeps the first QKV LDWEIGHTS fed.
        phase_load(0)
        phase_weights(("wq", "wk"))
        phase_load(1)
        phase_weights(("wv", "wo"))
        for phase in (phase_gn, phase_qkv, phase_st, phase_sr):
            for s in range(S):
                phase(s)
        for s in range(S):
            phase_ot(s)
            phase_o2(s)


_CACHE = {}


def make_in_maps(inputs):
    """Host-side sharding/layout prep shared by kernel() and the test/sim
    harnesses."""
    x = np.asarray(inputs["x"], dtype=np.float32)
    assert x.shape == (B, C, H, W)

    if COMPUTE == "bf16":
        import ml_dtypes
        wdt = ml_dtypes.bfloat16
    else:
        wdt = np.float32

    def wprep(w):
        # [o, c] -> transpose to [c, o] -> tile rows of 128 channels
        return np.ascontiguousarray(
            np.asarray(w, dtype=np.float32).T.reshape(CT, 128, C)
        ).astype(wdt)

    c0 = (np.asarray(inputs["wo"], dtype=np.float64)
          @ np.asarray(inputs["bv"], dtype=np.float64)
          + np.asarray(inputs["bo"], dtype=np.float64)).astype(np.float32)
    base = {
        "wq": wprep(inputs["wq"]), "wk": wprep(inputs["wk"]),
        "wv": wprep(inputs["wv"]), "wo": wprep(inputs["wo"]),
    }
    gmat = np.zeros((128, GPT), dtype=np.float32)
    gmt = np.zeros((GPT, 128), dtype=np.float32)
    for g in range(GPT):
        gmat[g * GSIZE:(g + 1) * GSIZE, g] = 1.0 / GSIZE
        gmt[g, g * GSIZE:(g + 1) * GSIZE] = 1.0
    cblob = np.zeros((128, 28), dtype=np.float32)
    for bi, arr in enumerate((inputs["bq"], inputs["bk"], c0,
                              inputs["gn_weight"], inputs["gn_bias"])):
        cblob[:, bi * CT:(bi + 1) * CT] = np.asarray(
            arr, dtype=np.float32).reshape(CT, 128).T
    cblob[:, 20:20 + GPT] = gmat
    base["cblob"] = cblob
    base["gmt"] = gmt
    base["ones2"] = np.ones((128, 128), dtype=wdt)
    xr = x.reshape(NCORES, S, CT, 128, N)
    return [dict(base, x=np.ascontiguousarray(xr[i])) for i in range(NCORES)]


def kernel(**inputs):
    if "nc" not in _CACHE:
        _CACHE["nc"] = build_nc()
    nc = _CACHE["nc"]

    in_maps = make_in_maps(inputs)
    res = run_bass_kernel_spmd(nc, in_maps, core_ids=list(range(NCORES)))

    out = np.empty((NCORES, S, CT, 128, N), dtype=np.float32)
    for i in range(NCORES):
        out[i] = res.results[i]["out"]
    return out.reshape(B, C, H, W)



# revision 11
# speedup vs baseline: 2.0228x; 2.0228x over previous
"""GroupNorm + single-head self-attention block (B=16, C=512, H=W=32) on 8
TRN2 NeuronCores.

Sharding: pure data-parallel over batch - 2 samples per core, no collectives.

Algebraic restructure vs the straightforward q/k/v pipeline (exact, done on
the host in f64):

  scores  = (Wq h)^T (Wk h) = h^T M h        with M  = Wq^T Wk
  o       = attn @ (V Wo^T)                  with WB = Wo @ Wv  (attn weights
            are scalars, so the output projection commutes into V)

so the device only runs TWO projection matmuls (G = M h and Vb = WB h)
instead of four (q, k, v, o2): 25% fewer PE columns and ~30% fewer PSUM
evacuations.  Biases stay exact:
  - bv, bo enter as c0 = Wo bv + bo, a per-channel constant added at the
    final (channel-major) evacuation;
  - bq, bk survive softmax only through the per-key term z[m] = (Wk^T bq) .
    h[:, m]; z is computed as one extra moving column riding the Vb matmul
    (whose output is pixel-major, so z lands per-partition) and folded into
    the exp bias. Per-query terms cancel in softmax.

Matmuls run in fp8-e4m3 with perf_mode=DoubleRow (2 contraction slices per
pass), weights pre-scaled x16 into the e4m3 sweet spot, exp shifted by -1.5
so E stays far below the TRN fp8 max of 240. fp32 PSUM accumulation.
Numpy-prototyped rel err vs the fp32 reference: 4.3e-3 (budget 2e-2).

Per-sample dataflow (C=512 channels, N=1024 pixels), channels-on-partitions:

  x    [128, CT=4, N]   f32
  GN:  bn_stats/bn_aggr per channel, group (16-ch) aggregation via a tiny
       matmul against a group-indicator matrix; h = a_c*x - b_c -> fp8
  G    [128, CT, N] = (M.T tiles).T @ h        (DR pairs over c')
  Vb   [128, MT=8, C] = h_tile.T @ WB.T tiles  (pixel-major; z column rides)
  ST   [m, n] = G_tile.T @ h                   (scores, transposed layout)
  E    = exp(ST*ISC + bias)  -> fp8            (bias = z/sqrt(C) - SHIFT)
  R    = 1/(16.T ones @ E)                     (softmax denominators, PE)
  OT   [c, n] = Vb_tile.T @ E                  (un-normalized attn output)
  y    = (OT*R + c0) + x                       (DVE mul in psum, Pool stt)
"""

import numpy as np

import concourse.bass as bass
import concourse.mybir as mybir
from concourse import tile
from concourse.bass_utils import run_bass_kernel_spmd


def _install_drain_patch():
    """This walrus build rejects Drain instructions carrying more than one
    semaphore wait (setupSyncWait<CTRL_NO_STRUCT>). Split the TileContext
    tail drain's waits across a chain of single-wait drains."""
    import concourse.tile as tile_mod
    from concourse.vector_clock import ScopedClock

    if getattr(tile_mod.TileContext, "_drain_patch_installed", False):
        return

    def _patched(self, tick_clock, wait_clock):
        nc = self.nc
        drain_inst = nc.sync.drain()
        wait_clock.add_sem_waits(
            drain_inst.ins, ScopedClock({None: tick_clock.global_clock})
        )
        si = drain_inst.ins.sync_info
        waits = list(si.on_wait or []) if si is not None else []
        if len(waits) > 1:
            si.on_wait = waits[:1]
            for w in waits[1:]:
                extra = nc.sync.drain()
                extra.ins.sync_info = mybir.SyncInfo(on_wait=[w], on_update=[])

        nc.all_engine_barrier()
        assert self.sems is not None
        popped = nc._tile_sem_poison_stack.pop()
        assert popped is self._sem_poison
        nc.clear_and_free_semaphores(list(self.sems.allocated().values()))
        nc.all_engine_barrier()

    tile_mod.TileContext._drain_and_barrier = _patched
    tile_mod.TileContext._drain_patch_installed = True


_install_drain_patch()

F32 = mybir.dt.float32
BF16 = mybir.dt.bfloat16
FP8 = mybir.dt.float8e4

B, C, H, W = 16, 512, 32, 32
N = H * W                      # 1024 pixels
NCORES = 8
S = B // NCORES                # samples per core
CT = C // 128                  # 4 channel tiles
MT = N // 128                  # 8 pixel tiles
NW = 512                       # matmul output chunk (PSUM bank = 512 f32)
NCH = N // NW                  # 2 chunks
GROUPS = 32
GSIZE = C // GROUPS            # 16 channels per group
GPT = 128 // GSIZE             # 8 groups per channel tile
EPS = 1e-5

COMPUTE = "fp8"                # "fp8" (DoubleRow) | "bf16"

CDT = {"fp8": FP8, "bf16": BF16}[COMPUTE]
SCL = 16.0 if COMPUTE == "fp8" else 1.0
SHIFT = 1.5
ISC = 1.0 / (SCL * float(np.sqrt(C)))
DR = mybir.MatmulPerfMode.DoubleRow if COMPUTE == "fp8" else None
KSTEP = 2 if COMPUTE == "fp8" else 1   # contraction subtiles per matmul


def _split_waits(nc, maxw=1):
    """This walrus build caps the number of sync waits an instruction can
    carry (varies by instruction class; Drain and Matmult/LDWEIGHTS observed
    failing). Hoist excess waits onto standalone EventSemaphore instructions
    inserted just before, on the same engine."""
    cnt = 0
    for f in nc.m.functions:
        for bb in f.blocks:
            insts = list(bb.instructions)
            out = []
            changed = False
            for inst in insts:
                si = inst.sync_info
                waits = list(si.on_wait) if (si is not None and si.on_wait) else []
                if len(waits) > maxw:
                    for w in waits[:-maxw]:
                        ev = mybir.InstEventSemaphore(
                            name=f"waitsplit_{cnt}", ins=[], outs=[])
                        cnt += 1
                        ev.engine = inst.engine
                        ev.sync_info = mybir.SyncInfo(on_wait=[w], on_update=[])
                        out.append(ev)
                    si.on_wait = waits[-maxw:]
                    changed = True
                out.append(inst)
            if changed:
                _replace_block_instructions(bb, out)
    return cnt


def _replace_block_instructions(bb, insts):
    try:
        bb.instructions = insts
        return
    except Exception:
        pass
    try:
        bb.instructions.clear()
        for i in insts:
            bb.instructions.append(i)
        return
    except Exception:
        pass
    raise RuntimeError("cannot rewrite block instructions")


def build_nc(split_waits=True):
    nc = bass.Bass(target_bir_lowering=False)

    x_ext = nc.declare_dram_parameter("x", [S, CT, 128, N], F32, isOutput=False)
    m8_ext = nc.declare_dram_parameter("m8t", [128, CT, C], CDT, isOutput=False)
    # wbt carries WB.T in cols 0:512 and the z-vector u in col 512 (+pad)
    wb_ext = nc.declare_dram_parameter("wbt8", [128, CT, C + 16], CDT,
                                       isOutput=False)
    # cblob columns: gnw[4] gnb[4] c0[4] gmat[8] -> [128, 20] f32
    cb_ext = nc.declare_dram_parameter("cblob", [128, 20], F32, isOutput=False)
    gmt_ext = nc.declare_dram_parameter("gmt", [GPT, 128], F32, isOutput=False)
    ones_ext = nc.declare_dram_parameter("ones16", [128, 2, 128], CDT,
                                         isOutput=False)
    out_ext = nc.declare_dram_parameter("out", [S, CT, 128, N], F32,
                                        isOutput=True)

    with tile.TileContext(nc) as tc:
        _body(nc, tc, x_ext, m8_ext, wb_ext, cb_ext, gmt_ext, ones_ext,
              out_ext)
    if split_waits:
        _split_waits(nc)
    return nc


def _mm_accum(nc, psum, lhsT3, rhs3, kparts):
    """Accumulating matmul over contraction subtiles. lhsT3/rhs3 are
    callables j -> AP: for fp8 they get slice(j, j+2) (DoubleRow pair),
    for bf16 slice(j, j+1)."""
    steps = list(range(0, kparts, KSTEP))
    for i, j in enumerate(steps):
        nc.tensor.matmul(
            psum,
            lhsT3(slice(j, j + KSTEP)),
            rhs3(slice(j, j + KSTEP)),
            start=(i == 0), stop=(i == len(steps) - 1),
            perf_mode=DR,
        )


def _body(nc, tc, x_ext, m8_ext, wb_ext, cb_ext, gmt_ext, ones_ext, out_ext):
    import contextlib

    ctx = contextlib.ExitStack()
    with ctx:
        consts = ctx.enter_context(tc.tile_pool(name="consts", bufs=1))
        sb = ctx.enter_context(tc.tile_pool(name="sb", bufs=1))
        ps = ctx.enter_context(tc.tile_pool(name="ps", space="PSUM", bufs=1))

        # ---------------- constants ----------------
        m8t = consts.tile([128, CT, C], CDT, tag="m8t")
        wbt = consts.tile([128, CT, C + 16], CDT, tag="wbt")

        cblob = consts.tile([128, 20], F32, tag="cblob")
        nc.gpsimd.dma_start(out=cblob, in_=cb_ext[:, :])
        b_sb = {}
        for bi, b in enumerate(("gnw", "gnb", "c0")):
            b_sb[b] = [cblob[:, bi * CT + ct:bi * CT + ct + 1]
                       for ct in range(CT)]
        gmat = cblob[:, 12:12 + GPT]

        ones16 = consts.tile([128, 2, 128], CDT, tag="ones16")
        nc.gpsimd.dma_start(out=ones16, in_=ones_ext[:, :, :])
        gmt = consts.tile([GPT, 128], F32, tag="gmt")
        nc.gpsimd.dma_start(out=gmt, in_=gmt_ext[:, :])
        eps_g = consts.tile([GPT, 1], F32, tag="eps_g")
        nc.vector.memset(eps_g, EPS)

        def warm_spin(k):
            warm = ps.tile([128, 128], F32, tag="small", bufs=2)
            for wi in range(k):
                if COMPUTE == "fp8":
                    nc.tensor.matmul(warm, ones16[:, 0:2, :], ones16[:, 0:2, :],
                                     start=(wi == 0), stop=(wi == k - 1),
                                     perf_mode=DR)
                else:
                    nc.tensor.matmul(warm, ones16[:, 0, :], ones16[:, 0, :],
                                     start=(wi == 0), stop=(wi == k - 1))

        # ---------------- per-sample pipeline phases ----------------
        st = [dict() for _ in range(S)]

        def phase_load(s):
            xt = sb.tile([128, CT, N], F32, name=f"x{s}", tag="x", bufs=2)
            for ct in range(CT):
                for q in range(2):
                    nc.sync.dma_start(out=xt[:, ct, q * 512:(q + 1) * 512],
                                      in_=x_ext[s, ct, :, q * 512:(q + 1) * 512])
            st[s]["x"] = xt

        def phase_gn(s):
            xt = st[s]["x"]
            stats3 = []
            for ct in range(CT):
                s3 = sb.tile([128, 3], F32, tag=f"s3_{ct}", bufs=2)
                if ct < 2:
                    # DVE path: bn_stats -> [mean, var], plus mean^2
                    st6 = sb.tile([128, 2, 6], F32, tag="st6", bufs=4)
                    nc.vector.bn_stats(out=st6[:, 0, :], in_=xt[:, ct, 0:512])
                    nc.vector.bn_stats(out=st6[:, 1, :], in_=xt[:, ct, 512:1024])
                    nc.vector.bn_aggr(out=s3[:, 0:2], in_=st6)
                    nc.vector.tensor_mul(out=s3[:, 2:3], in0=s3[:, 0:1],
                                         in1=s3[:, 0:1])
                else:
                    # ACT path: accum_out sums along the free axis.
                    # col0 = mean, col1 = E[x^2], col2 = 0. Downstream uses
                    # col1+col2 = E[x^2] = var + mean^2, same as the DVE path.
                    scr = sb.tile([128, N], CDT, tag="gnscr", bufs=2)
                    nc.scalar.activation(
                        out=scr, in_=xt[:, ct, :],
                        func=mybir.ActivationFunctionType.Copy,
                        scale=1.0 / N, accum_out=s3[:, 0:1])
                    nc.scalar.activation(
                        out=scr, in_=xt[:, ct, :],
                        func=mybir.ActivationFunctionType.Square,
                        scale=1.0 / float(np.sqrt(N)), accum_out=s3[:, 1:2])
                    nc.vector.memset(s3[:, 2:3], 0.0)
                stats3.append(s3)

            ab = []
            for ct in range(CT):
                gp = ps.tile([GPT, 3], F32, tag="small", bufs=2)
                nc.tensor.matmul(gp, gmat, stats3[ct], start=True, stop=True)
                gs = sb.tile([GPT, 3], F32, tag="gs", bufs=4)
                nc.vector.tensor_copy(out=gs, in_=gp)
                # var_g = (E[var] + E[mean^2]) - (E[mean])^2
                m2 = sb.tile([GPT, 3], F32, tag="m2", bufs=4)
                nc.vector.tensor_add(out=m2[:, 1:2], in0=gs[:, 1:2],
                                     in1=gs[:, 2:3])
                nc.vector.tensor_mul(out=m2[:, 0:1], in0=gs[:, 0:1],
                                     in1=gs[:, 0:1])
                nc.vector.tensor_sub(out=m2[:, 2:3], in0=m2[:, 1:2],
                                     in1=m2[:, 0:1])
                # rsqrt(var+eps) = exp(-0.5*ln(var+eps)): Ln/Exp share one
                # ACT table set with Copy/Square/Identity (no table reloads)
                s2 = sb.tile([GPT, 2], F32, tag="s2", bufs=4)
                nc.scalar.activation(out=s2[:, 1:2], in_=m2[:, 2:3],
                                     func=mybir.ActivationFunctionType.Ln,
                                     bias=eps_g, scale=1.0)
                nc.scalar.activation(out=s2[:, 1:2], in_=s2[:, 1:2],
                                     func=mybir.ActivationFunctionType.Exp,
                                     scale=-0.5)
                nc.gpsimd.tensor_copy(out=s2[:, 0:1], in_=gs[:, 0:1])

                abp = ps.tile([128, 2], F32, tag="small", bufs=2)
                nc.tensor.matmul(abp, gmt, s2, start=True, stop=True)
                a_c = sb.tile([128, 1], F32, tag=f"a_{ct}", bufs=2)
                nc.vector.tensor_mul(out=a_c, in0=abp[:, 1:2],
                                     in1=b_sb["gnw"][ct])
                bneg = sb.tile([128, 1], F32, tag=f"bneg_{ct}", bufs=2)
                nc.vector.scalar_tensor_tensor(
                    out=bneg, in0=abp[:, 0:1], scalar=a_c,
                    in1=b_sb["gnb"][ct],
                    op0=mybir.AluOpType.mult, op1=mybir.AluOpType.subtract,
                )
                ab.append((a_c, bneg))
            st[s]["ab"] = ab

        def phase_h(s):
            xt = st[s]["x"]
            ab = st[s]["ab"]
            ht = sb.tile([128, CT, N], CDT, name=f"h{s}", tag="h", bufs=2)
            for ct in range(CT):
                a_c, bneg = ab[ct]
                if ct < 2:
                    nc.vector.tensor_scalar(
                        out=ht[:, ct, :], in0=xt[:, ct, :], scalar1=a_c,
                        scalar2=bneg,
                        op0=mybir.AluOpType.mult, op1=mybir.AluOpType.subtract,
                    )
                else:
                    # Pool can't write fp8; ACT computes a*x + (-bneg)
                    bpos = sb.tile([128, 1], F32, tag=f"bp_{ct}", bufs=2)
                    nc.gpsimd.tensor_scalar_mul(out=bpos, in0=bneg,
                                                scalar1=-1.0)
                    nc.scalar.activation(
                        out=ht[:, ct, :], in_=xt[:, ct, :],
                        func=mybir.ActivationFunctionType.Identity,
                        scale=a_c, bias=bpos,
                    )
            st[s]["h"] = ht

        def phase_g(s):
            # G = M8 @ h: stationary = m8t pair-slice, moving = h
            ht = st[s]["h"]
            gt = sb.tile([128, CT, N], CDT, name=f"g{s}", tag="g", bufs=2)
            for ot in range(CT):
                for nch in range(NCH):
                    pp = ps.tile([128, NW], F32, tag="mm", bufs=5)
                    _mm_accum(
                        nc, pp,
                        lambda j: m8t[:, j, ot * 128:(ot + 1) * 128],
                        lambda j: ht[:, j, nch * NW:(nch + 1) * NW],
                        CT)
                    nc.scalar.copy(out=gt[:, ot, nch * NW:(nch + 1) * NW],
                                   in_=pp)
            st[s]["g"] = gt

        def phase_vb(s):
            # Vb[m, o] = h_tile.T @ WB.T; z rides as moving column 512.
            ht = st[s]["h"]
            vt = sb.tile([128, MT, C], CDT, name=f"v{s}", tag="v", bufs=2)
            zps = ps.tile([128, MT], F32, tag="small", bufs=2)
            for mt in range(MT):
                vp = ps.tile([128, NW], F32, tag="mm", bufs=5)
                _mm_accum(
                    nc, vp,
                    lambda j: ht[:, j, mt * 128:(mt + 1) * 128],
                    lambda j: wbt[:, j, 0:C],
                    CT)
                # z[m] column: same stationary, 1-col moving
                _mm_accum(
                    nc, zps[:, mt:mt + 1],
                    lambda j: ht[:, j, mt * 128:(mt + 1) * 128],
                    lambda j: wbt[:, j, C:C + 1],
                    CT)
                nc.vector.tensor_copy(out=vt[:, mt, :], in_=vp)
            # exp bias: z*ISC - SHIFT  (per-partition per-mt)
            bias_t = sb.tile([128, MT], F32, name=f"bias{s}", tag="bias",
                             bufs=2)
            nc.vector.tensor_scalar(
                out=bias_t, in0=zps, scalar1=ISC, scalar2=SHIFT,
                op0=mybir.AluOpType.mult, op1=mybir.AluOpType.subtract,
            )
            st[s]["v"], st[s]["bias"] = vt, bias_t

        def phase_st(s, nch):
            # ST = G_tile.T @ h ; exp -> E fp8
            ht, gt, bias_t = st[s]["h"], st[s]["g"], st[s]["bias"]
            if "e" not in st[s]:
                st[s]["e"] = sb.tile([128, MT, N], CDT, name=f"e{s}", tag="e",
                                     bufs=2)
            et = st[s]["e"]
            for mt in range(MT):
                sp = ps.tile([128, NW], F32, tag="mm", bufs=5)
                _mm_accum(
                    nc, sp,
                    lambda j: gt[:, j, mt * 128:(mt + 1) * 128],
                    lambda j: ht[:, j, nch * NW:(nch + 1) * NW],
                    CT)
                nc.scalar.activation(
                    out=et[:, mt, nch * NW:(nch + 1) * NW], in_=sp,
                    func=mybir.ActivationFunctionType.Exp,
                    bias=bias_t[:, mt:mt + 1], scale=ISC,
                )

        def phase_r(s, nch):
            # R = 1 / (SCL * colsum(E)) via ones(SCL) matmul + reciprocal
            et = st[s]["e"]
            if "r" not in st[s]:
                st[s]["r"] = sb.tile([128, N], F32, name=f"r{s}", tag="r",
                                     bufs=2)
            rt = st[s]["r"]
            srp = ps.tile([128, NW], F32, tag="mm", bufs=5)
            steps = list(range(0, MT, KSTEP))
            for i, k in enumerate(steps):
                nc.tensor.matmul(
                    srp, ones16[:, 0:KSTEP, :],
                    et[:, k:k + KSTEP, nch * NW:(nch + 1) * NW],
                    start=(i == 0), stop=(i == len(steps) - 1),
                    perf_mode=DR)
            nc.vector.reciprocal(out=rt[:, nch * NW:(nch + 1) * NW], in_=srp)

        def phase_ot(s, nch):
            # OT = Vb_tile.T @ E ; y = (OT*R + c0) + x, in place over x
            xt, vt, et, rt = st[s]["x"], st[s]["v"], st[s]["e"], st[s]["r"]
            for ct in range(CT):
                op_ = ps.tile([128, NW], F32, tag="mm", bufs=5)
                steps = list(range(0, MT, KSTEP))
                for i, k in enumerate(steps):
                    nc.tensor.matmul(
                        op_, vt[:, k:k + KSTEP, ct * 128:(ct + 1) * 128],
                        et[:, k:k + KSTEP, nch * NW:(nch + 1) * NW],
                        start=(i == 0), stop=(i == len(steps) - 1),
                        perf_mode=DR)
                tmp = sb.tile([128, NW], F32, tag="otmp", bufs=4)
                nc.vector.tensor_mul(
                    out=tmp, in0=op_, in1=rt[:, nch * NW:(nch + 1) * NW])
                # residual add on Pool (SBUF-only); c0 is added on the host
                nc.gpsimd.tensor_add(
                    out=xt[:, ct, nch * NW:(nch + 1) * NW], in0=tmp,
                    in1=xt[:, ct, nch * NW:(nch + 1) * NW],
                )
                nc.sync.dma_start(
                    out=out_ext[s, ct, :, nch * NW:(nch + 1) * NW],
                    in_=xt[:, ct, nch * NW:(nch + 1) * NW])

        # ---------------- emission ----------------
        warm_spin(8)
        phase_load(0)
        nc.sync.dma_start(out=m8t, in_=m8_ext[:, :, :])
        phase_load(1)
        nc.sync.dma_start(out=wbt, in_=wb_ext[:, :, :])

        phase_gn(0)
        warm_spin(8)
        phase_gn(1)
        warm_spin(8)
        for s in range(S):
            phase_h(s)
        for s in range(S):
            phase_g(s)
            phase_vb(s)
        phase_st(0, 0)
        phase_st(0, 1)
        phase_r(0, 0)
        phase_ot(0, 0)
        phase_st(1, 0)
        phase_r(0, 1)
        phase_ot(0, 1)
        phase_st(1, 1)
        phase_r(1, 0)
        phase_ot(1, 0)
        phase_r(1, 1)
        phase_ot(1, 1)


_CACHE = {}


def make_in_maps(inputs):
    """Host-side sharding/layout prep shared by kernel() and the test/sim
    harnesses."""
    import ml_dtypes

    x = np.asarray(inputs["x"], dtype=np.float32)
    assert x.shape == (B, C, H, W)

    wdt = ml_dtypes.float8_e4m3fn if COMPUTE == "fp8" else ml_dtypes.bfloat16

    def q(a):
        return np.clip(a * SCL, -240.0, 240.0).astype(wdt)

    wq = np.asarray(inputs["wq"], dtype=np.float64)
    wk = np.asarray(inputs["wk"], dtype=np.float64)
    wv = np.asarray(inputs["wv"], dtype=np.float64)
    wo = np.asarray(inputs["wo"], dtype=np.float64)
    bq = np.asarray(inputs["bq"], dtype=np.float64)
    bv = np.asarray(inputs["bv"], dtype=np.float64)
    bo = np.asarray(inputs["bo"], dtype=np.float64)

    M8 = wq.T @ wk                 # [c, c']
    WB = wo @ wv                   # [o, c]
    u = wk.T @ bq                  # [c'] per-key softmax bias vector
    c0 = (wo @ bv + bo).astype(np.float32)

    # m8t[p, j, o] = M8[o, j*128+p]  (stationary tiles of M8^T)
    m8t = q(np.ascontiguousarray(
        M8.T.reshape(CT, 128, C).transpose(1, 0, 2)))
    # wbt[p, j, 0:512] = WB[o, j*128+p]^T tiles; col 512 = u; pad to 528
    wbt = np.zeros((128, CT, C + 16), dtype=np.float64)
    wbt[:, :, 0:C] = WB.T.reshape(CT, 128, C).transpose(1, 0, 2)
    wbt[:, :, C] = u.reshape(CT, 128).T
    wbt = q(wbt)

    gmat = np.zeros((128, GPT), dtype=np.float32)
    gmt = np.zeros((GPT, 128), dtype=np.float32)
    for g in range(GPT):
        gmat[g * GSIZE:(g + 1) * GSIZE, g] = 1.0 / GSIZE
        gmt[g, g * GSIZE:(g + 1) * GSIZE] = 1.0
    cblob = np.zeros((128, 20), dtype=np.float32)
    for bi, arr in enumerate((inputs["gn_weight"], inputs["gn_bias"], c0)):
        cblob[:, bi * CT:(bi + 1) * CT] = np.asarray(
            arr, dtype=np.float32).reshape(CT, 128).T
    cblob[:, 12:12 + GPT] = gmat

    base = {
        "m8t": m8t, "wbt8": wbt, "cblob": cblob, "gmt": gmt,
        "ones16": np.full((128, 2, 128), SCL, dtype=wdt),
    }
    xr = x.reshape(NCORES, S, CT, 128, N)
    return [dict(base, x=np.ascontiguousarray(xr[i])) for i in range(NCORES)]


def kernel(**inputs):
    if "nc" not in _CACHE:
        _CACHE["nc"] = build_nc()
    nc = _CACHE["nc"]

    in_maps = make_in_maps(inputs)
    res = run_bass_kernel_spmd(nc, in_maps, core_ids=list(range(NCORES)))

    out = np.empty((NCORES, S, CT, 128, N), dtype=np.float32)
    for i in range(NCORES):
        out[i] = res.results[i]["out"]
    out = out.reshape(B, C, H, W)
    # c0 = Wo bv + bo enters the output as a per-channel constant
    # (softmax rows sum to 1); zero for zero biases.
    c0 = (np.asarray(inputs["wo"], dtype=np.float64)
          @ np.asarray(inputs["bv"], dtype=np.float64)
          + np.asarray(inputs["bo"], dtype=np.float64)).astype(np.float32)
    if np.any(c0):
        out += c0[None, :, None, None]
    return out
